# revision 1
# baseline (speedup 1.0000x reference)
"""Trainium2 Bass kernel for nn_Attention_64063732187236.

Reference computation (B=4, N=2048, DIM=512, HEADS=8, DIM_HEAD=64):
    qkv = x @ w_qkv ; q,k,v = split(qkv) -> [b,h,n,64]
    atten = softmax((q * HEADS**-0.5) @ k^T + drop_mask * -1e12)
    out   = (atten @ v) reshaped -> [b,n,512] @ w_out

Sharding: 8 cores = 4 batches x 2 head-groups (4 heads each).
Each core computes a partial output  x[b] -> attn(heads hg..hg+3) @ w_out[rows]
and the host sums the two head-group partials per batch.

On-core layout is fully "transposed": scores are computed as
S^T[k,q] = K Q^T so that the PV matmul contracts over k naturally, and the
out-projection consumes attn^T directly.  Softmax skips the max-subtraction
(scores are O(10), exp is safe in fp32) and gets the row-sum for free from a
ones-column appended to V.  Mask is applied post-exp as P *= (1-mask).
"""

import numpy as np

import concourse.bass as bass
import concourse.bacc as bacc
import concourse.tile as tile
from concourse import mybir
from concourse.bass_utils import run_bass_kernel_spmd

F32 = mybir.dt.float32
BF16 = mybir.dt.bfloat16
NP_BF16 = mybir.dt.np(BF16)

# Full-size problem constants
B, N, D = 4, 2048, 512
HEADS, DH = 8, 64
HL = 4               # heads per core (local)
GROUPS = HEADS // HL  # head groups = 2
SCALE = float(HEADS) ** -0.5   # reference quirk: scales by heads, not dim_head


def build_graph(nc, N=N, D=D, HL=HL):
    """Emit the per-core attention graph into `nc` (inside a TileContext)."""
    KT = N // 128          # key tiles
    NT = N // 128          # seq tiles
    DC = D // 128          # d-model chunks
    HP = (HL * DH) // 128  # head-pair chunks (2 for HL=4)
    QW = min(1024, N)      # ACT exp width per op
    NQ = N // QW

    xT = nc.dram_tensor("xT", [D, N], BF16, kind="ExternalInput").ap()
    wqkv = nc.dram_tensor("wqkv", [D, 3 * HL * DH], BF16, kind="ExternalInput").ap()
    wout = nc.dram_tensor("wout", [HL * DH, D], BF16, kind="ExternalInput").ap()
    nmaskT = nc.dram_tensor("nmaskT", [HL, N, N], BF16, kind="ExternalInput").ap()
    out = nc.dram_tensor("out", [N, D], F32, kind="ExternalOutput").ap()
    rscratch = [nc.dram_tensor(f"rscratch{h}", [1, N], F32).ap() for h in range(HL)]

    tc = nc.tc  # set by caller

    with tc.tile_pool(name="wts", bufs=1) as wts, \
         tc.tile_pool(name="persist", bufs=1) as persist:

        # ---- Phase A: inputs arrive pre-cast to bf16; plain DMA loads ----
        xTb = []
        wb = []
        woutb = []
        for dc in range(DC):
            t = wts.tile([128, 3 * HL * DH], BF16, tag=f"wb{dc}", name=f"wb{dc}")
            nc.sync.dma_start(out=t, in_=wqkv[dc * 128:(dc + 1) * 128, :])
            wb.append(t)
        for dc in range(DC):
            t = wts.tile([128, N], BF16, tag=f"xtb{dc}", name=f"xtb{dc}")
            eng = nc.gpsimd if dc % 2 == 0 else nc.sync
            eng.dma_start(out=t, in_=xT[dc * 128:(dc + 1) * 128, :])
            xTb.append(t)
        for c in range(HP):
            t = wts.tile([128, D], BF16, tag=f"wob{c}", name=f"wob{c}")
            nc.sync.dma_start(out=t, in_=wout[c * 128:(c + 1) * 128, :])
            woutb.append(t)

        # ---- Phase B: projections q^T, k^T (head-pair packed) and V(+ones) ----
        qTb = [persist.tile([128, N], BF16, tag=f"qT{p}", name=f"qT{p}") for p in range(HP)]
        kTb = [persist.tile([128, N], BF16, tag=f"kT{p}", name=f"kT{p}") for p in range(HP)]
        vplus = [persist.tile([128, HL, DH + 1], BF16, tag=f"vp{t}", name=f"vp{t}")
                 for t in range(NT)]
        with tc.tile_pool(name="psB", bufs=2, space="PSUM") as psB:
            voff = 2 * HL * DH

            def qk_proj(which, dst, hp, half=None):
                off = which * HL * DH
                halves = range((N + 1023) // 1024) if half is None else [half]
                for half in halves:
                    w = min(1024, N - half * 1024)
                    ps = psB.tile([128, w], F32, tag="qk", name="psqk")
                    for dc in range(DC):
                        for s0 in range(0, w, 512):
                            sw = min(512, w - s0)
                            nc.tensor.matmul(
                                ps[:, s0:s0 + sw],
                                lhsT=wb[dc][:, off + hp * 128: off + (hp + 1) * 128],
                                rhs=xTb[dc][:, half * 1024 + s0: half * 1024 + s0 + sw],
                                start=(dc == 0), stop=(dc == DC - 1))
                    nc.scalar.copy(
                        dst[hp][:, half * 1024: half * 1024 + w], ps)

            # pair 0 first, q/k interleaved by half, so head-0 attention on
            # the first qn half can begin after just two projection rounds
            qk_proj(0, qTb, 0, half=0)
            qk_proj(1, kTb, 0, half=0)
            qk_proj(0, qTb, 0, half=1)
            qk_proj(1, kTb, 0, half=1)
            for nt in range(NT):
                psv = psB.tile([128, HL * DH], F32, tag="v")
                for dc in range(DC):
                    nc.tensor.matmul(
                        psv,
                        lhsT=xTb[dc][:, nt * 128:(nt + 1) * 128],
                        rhs=wb[dc][:, voff: voff + HL * DH],
                        start=(dc == 0), stop=(dc == DC - 1))
                nc.vector.memset(vplus[nt], 1.0)
                nc.vector.tensor_copy(
                    vplus[nt][:, :, 0:DH],
                    psv.rearrange("p (h d) -> p h d", h=HL))
            for hp in range(1, HP):
                qk_proj(0, qTb, hp)
                qk_proj(1, kTb, hp)

        # ---- Phase C: attention per local head + interleaved out-proj ----
        attnT = persist.tile([128, HP, N], BF16, tag="attnT")
        with tc.tile_pool(name="psS", bufs=2, space="PSUM") as psS, \
             tc.tile_pool(name="psPV", bufs=1, space="PSUM") as psPV, \
             tc.tile_pool(name="pmask", bufs=8) as pmask, \
             tc.tile_pool(name="pprob", bufs=9) as pprob, \
             tc.tile_pool(name="psmall", bufs=3) as psmall:
            for h in range(HL):
                hp, ho = h // 2, (h % 2) * 64
                pv = psPV.tile([DH + 1, N], F32, tag="pv")

                def pv_mms(kt, pt):
                    for s0 in range(0, N, 512):
                        sw = min(512, N - s0)
                        nc.tensor.matmul(
                            pv[:, s0:s0 + sw],
                            lhsT=vplus[kt][:, h, :],
                            rhs=pt[:, s0:s0 + sw],
                            start=(kt == 0), stop=(kt == KT - 1))

                # kt loop is software-pipelined: PV matmuls lag one iteration
                # so the PE stream never blocks on the exp/mask of the same kt.
                prev = None
                for kt in range(KT):
                    nm = pmask.tile([128, N], BF16, tag="nm")
                    nc.sync.dma_start(
                        out=nm, in_=nmaskT[h, kt * 128:(kt + 1) * 128, :])
                    if h < HL - 1:
                        pt = pprob.tile([128, N], BF16, tag="pt", name="pt")
                    else:
                        pt = pprob.tile([128, N], BF16, tag="ptl", bufs=3,
                                        name="ptl")
                    with tc.high_priority(offset=150):
                        for qh in range(NQ):
                            s = psS.tile([128, QW], F32, tag="s")
                            for s0 in range(0, QW, 512):
                                sw = min(512, QW - s0)
                                nc.tensor.matmul(
                                    s[:, s0:s0 + sw],
                                    lhsT=kTb[hp][ho:ho + 64, kt * 128:(kt + 1) * 128],
                                    rhs=qTb[hp][ho:ho + 64, qh * QW + s0: qh * QW + s0 + sw],
                                    start=True, stop=True)
                            nc.scalar.activation(
                                pt[:, qh * QW:(qh + 1) * QW], s,
                                mybir.ActivationFunctionType.Exp, scale=SCALE)
                    nc.vector.tensor_mul(pt, pt, nm)
                    if prev is not None:
                        pv_mms(*prev)
                    prev = (kt, pt)
                pv_mms(*prev)
                # Copy pv to SBUF right away (frees PSUM for the next head),
                # then normalize asynchronously: attnT[h] = pv[0:64]/rowsum.
                # rowsum row -> [128, N/128] so reciprocal uses all lanes,
                # then broadcast to 64 partitions via a DRAM-bounce DMA.
                if h < HL - 1:
                    pvs = psmall.tile([DH + 1, N], F32, tag="pvs")
                    nc.vector.tensor_copy(pvs, pv)
                    rsq = psmall.tile([128, N // 128], F32, tag="rsq")
                    nc.sync.dma_start(out=rsq, in_=pvs[DH:DH + 1, :])
                    nc.vector.reciprocal(rsq, rsq)
                    nc.sync.dma_start(out=rscratch[h], in_=rsq)
                    rb = psmall.tile([64, N], F32, tag="rb")
                    rb_src = bass.AP(
                        tensor=rscratch[h].tensor, offset=rscratch[h].offset,
                        ap=[[0, 64]] + list(rscratch[h].ap[1:]))
                    nc.sync.dma_start(out=rb, in_=rb_src)
                    nc.vector.tensor_mul(attnT[ho:ho + 64, hp, :], pvs[0:DH, :], rb)
                else:
                    # last head: normalize + out-projection pipelined by halves
                    # of the sequence so nothing waits on the full-row chain
                    HN = N // 2
                    pvs = psmall.tile([DH + 1, N], F32, tag="pvs")
                    for hh in range(2):
                        hs = slice(hh * HN, (hh + 1) * HN)
                        nc.scalar.copy(pvs[DH:DH + 1, hs], pv[DH:DH + 1, hs])
                        nc.vector.tensor_copy(pvs[0:DH, hs], pv[0:DH, hs])
                        rsq = psmall.tile([128, HN // 128], F32, tag="rsq")
                        nc.sync.dma_start(out=rsq, in_=pvs[DH:DH + 1, hs])
                        nc.vector.reciprocal(rsq, rsq)
                        rsc = rscratch[h][0:1, hs]
                        nc.sync.dma_start(out=rsc, in_=rsq)
                        rb = psmall.tile([64, HN], F32, tag="rb")
                        rb_src = bass.AP(
                            tensor=rsc.tensor, offset=rsc.offset,
                            ap=[[0, 64]] + list(rsc.ap[1:]))
                        nc.sync.dma_start(out=rb, in_=rb_src)
                        NC4 = N // 4
                        for qq in range(2 * hh, 2 * hh + 2):
                            nc.vector.tensor_mul(
                                attnT[ho:ho + 64, hp, qq * NC4:(qq + 1) * NC4],
                                pvs[0:DH, qq * NC4:(qq + 1) * NC4],
                                rb[:, qq * NC4 - hh * HN:(qq + 1) * NC4 - hh * HN])
                            for nt in range(qq * NT // 4, (qq + 1) * NT // 4):
                                po = psS.tile([128, D], F32, tag="s", name="po2")
                                for c in range(HP):
                                    nc.tensor.matmul(
                                        po, lhsT=attnT[:, c, nt * 128:(nt + 1) * 128],
                                        rhs=woutb[c], start=(c == 0),
                                        stop=(c == HP - 1))
                                ob = psmall.tile([128, D], F32, tag="ob", bufs=6)
                                nc.scalar.copy(ob, po)
                                eng = nc.sync if nt % 2 == 0 else nc.gpsimd
                                eng.dma_start(
                                    out=out[nt * 128:(nt + 1) * 128, :], in_=ob)


def build_bass(N=N, D=D, HL=HL):
    nc = bacc.Bacc("TRN2", target_bir_lowering=False, debug=False, num_devices=8)
    with tile.TileContext(nc) as tc:
        nc.tc = tc
        build_graph(nc, N=N, D=D, HL=HL)
    nc.compile()
    return nc


def shard_inputs(x, drop_mask, w_qkv, w_out):
    """Host-side sharding: returns in_maps for the 8 cores."""
    x = np.asarray(x, dtype=np.float32)
    drop_mask = np.asarray(drop_mask)
    w_qkv = np.asarray(w_qkv, dtype=np.float32)
    w_out = np.asarray(w_out, dtype=np.float32)
    inner = HEADS * DH
    in_maps = []
    for c in range(8):
        b, g = c // GROUPS, c % GROUPS
        cols = slice(g * HL * DH, (g + 1) * HL * DH)
        wq = w_qkv[:, cols]
        wk = w_qkv[:, inner:][:, cols]
        wv = w_qkv[:, 2 * inner:][:, cols]
        nmT = np.empty((HL, N, N), dtype=NP_BF16)
        for hh in range(HL):
            nmT[hh] = (~drop_mask[b, g * HL + hh]).T.astype(NP_BF16)
        in_maps.append({
            "xT": np.ascontiguousarray(x[b].T).astype(NP_BF16),
            "wqkv": np.ascontiguousarray(
                np.concatenate([wq, wk, wv], axis=1)).astype(NP_BF16),
            "wout": np.ascontiguousarray(
                w_out[g * HL * DH:(g + 1) * HL * DH, :]).astype(NP_BF16),
            "nmaskT": nmT,
        })
    return in_maps


_CACHED_NC = None


def _get_nc():
    global _CACHED_NC
    if _CACHED_NC is None:
        _CACHED_NC = build_bass()
    return _CACHED_NC


def kernel(x, drop_mask, w_qkv, w_out, _trace=False):
    nc = _get_nc()
    in_maps = shard_inputs(x, drop_mask, w_qkv, w_out)
    res = run_bass_kernel_spmd(nc, in_maps, core_ids=list(range(8)), trace=_trace)
    outs = [np.asarray(r["out"], dtype=np.float32) for r in res.results]
    full = np.empty((B, N, D), dtype=np.float32)
    for b in range(B):
        full[b] = outs[b * GROUPS]
        for g in range(1, GROUPS):
            full[b] += outs[b * GROUPS + g]
    kernel.last_results = res
    return full



# revision 105
# speedup vs baseline: 1.1903x; 1.1903x over previous
"""Trainium2 Bass kernel for nn_Attention_64063732187236.

Reference computation (B=4, N=2048, DIM=512, HEADS=8, DIM_HEAD=64):
    qkv = x @ w_qkv ; q,k,v = split(qkv) -> [b,h,n,64]
    atten = softmax((q * HEADS**-0.5) @ k^T + drop_mask * -1e12)
    out   = (atten @ v) reshaped -> [b,n,512] @ w_out

Sharding: 8 cores = 4 batches x 2 head-groups (4 heads each).
Each core computes a partial output  x[b] -> attn(heads hg..hg+3) @ w_out[rows]
and the host sums the two head-group partials per batch (bf16 partials).

v3 design (cost-model driven):
- PE runs the bf16 matmul floor (~136.5us/core: scores 54.6 + PV 54.6 +
  projections 27.3); every other engine is kept strictly below it
  (ACT exp-only ~133us, DVE ~120us, DMA ~108us, Pool light).
- One flat 128-slot software pipeline over (head, column-half, kt):
  scores [128,1024] -> exp (ACT; 8 mid-stream chunks use a bf16
  Schraudolph bitcast-exp on DVE instead, trading ~0.05% output error
  for decoupling the co-critical ACT engine) -> mask-mul (DVE at the
  0.5 bf16 rate, high priority so PV never waits) -> PV lagged 13 slots
  so the pipeline crosses half boundaries without stalling on the PSUM
  pv tile (psS 2x[128,1024] + pv [65,1024] + proj/out buffer = 8 banks;
  psB doubles as a third scores buffer once the fillers are done).
- QKV projections are cut into ~0.4-0.9us units and fed into the PE
  stream as filler exactly where the data dependencies allow. NOTE: in
  the Tile framework, program order defines read/write semantics - every
  consumer must be emitted after its producer (the po-ride is gated on
  the h3-A finisher's emission for exactly this reason).
- Rowsum via a ones-column in V (M=65 costs nothing); normalize =
  reciprocal (DVE) + gpsimd partition_broadcast + multiply (Pool for
  mid-kernel halves, DVE for the last head).
- Out-projection rides inside the last head's second half; the final
  tail is pipelined in 256-column chunks across ACT/DVE/Pool/PE, with
  bf16 partial outputs summed on the host.
"""

import numpy as np

import concourse.bass as bass
import concourse.bacc as bacc
import concourse.tile as tile
from concourse import mybir
from concourse.bass_utils import run_bass_kernel_spmd

F32 = mybir.dt.float32
BF16 = mybir.dt.bfloat16
NP_BF16 = mybir.dt.np(BF16)

# Full-size problem constants
B, N, D = 4, 2048, 512
HEADS, DH = 8, 64
HL = 4                # heads per core (local)
GROUPS = HEADS // HL  # head groups = 2
HP = HL // 2          # packed head pairs (qT/kT rows = 2 heads x 64 dims)
KT = N // 128         # key tiles
NT = N // 128         # seq tiles
DC = D // 128         # d-model chunks
HALF = N // 2
SCALE = float(HEADS) ** -0.5   # reference quirk: scales by heads, not dim_head
Exp = mybir.ActivationFunctionType.Exp
I16 = mybir.dt.int16
# bf16 Schraudolph exp constants: bitcast(round(x*A16+B16) as i16) ~= e^x
# (1.8% rms / 4.5% max per element; used on 8 of 128 chunks to offload the
# co-critical ACT engine onto DVE slack -- output-level impact ~0.1%)
A16 = 128.0 / float(np.log(2.0))
B16 = 127.0 * 128 - 8


def build_graph(nc):
    tc = nc.tc

    xT = nc.dram_tensor("xT", [D, N], BF16, kind="ExternalInput").ap()
    wqkv = nc.dram_tensor("wqkv", [D, 3 * HL * DH], BF16, kind="ExternalInput").ap()
    wout = nc.dram_tensor("wout", [HL * DH, D], BF16, kind="ExternalInput").ap()
    nmaskT = nc.dram_tensor("nmaskT", [HL, N, N], BF16, kind="ExternalInput").ap()
    out = nc.dram_tensor("out", [N, D], BF16, kind="ExternalOutput").ap()

    with tc.tile_pool(name="wts", bufs=1) as wts, \
         tc.tile_pool(name="persist", bufs=1) as persist, \
         tc.tile_pool(name="pmask", bufs=22) as pmask, \
         tc.tile_pool(name="pprob", bufs=15) as pprob, \
         tc.tile_pool(name="psmall", bufs=2) as psmall, \
         tc.tile_pool(name="pout", bufs=4) as pout:

        # ---- input tiles; xT split in column halves for a faster start ----
        wb = [wts.tile([128, 3 * HL * DH], BF16, tag=f"wb{dc}", name=f"wb{dc}")
              for dc in range(DC)]
        xTb = [wts.tile([128, N], BF16, tag=f"xtb{dc}", name=f"xtb{dc}")
               for dc in range(DC)]
        woutb = [wts.tile([128, D], BF16, tag=f"wob{c}", name=f"wob{c}")
                 for c in range(HP)]
        wdum = wts.tile([128, 512], BF16, tag="wdum", name="wdum")
        nc.vector.memset(wdum, 0.0)

        for dc in range(DC):
            nc.sync.dma_start(out=wb[dc], in_=wqkv[dc * 128:(dc + 1) * 128, :])
            nc.sync.dma_start(out=xTb[dc][:, 0:512],
                              in_=xT[dc * 128:(dc + 1) * 128, 0:512])
            nc.gpsimd.dma_start(out=xTb[dc][:, 512:HALF],
                                in_=xT[dc * 128:(dc + 1) * 128, 512:HALF])
        for c in range(HP):
            nc.sync.dma_start(out=woutb[c], in_=wout[c * 128:(c + 1) * 128, :])
        for dc in range(DC):
            nc.gpsimd.dma_start(out=xTb[dc][:, HALF:N],
                                in_=xT[dc * 128:(dc + 1) * 128, HALF:N])

        # mask tiles: nm[h][kt] = [128 keys, N queries] bf16 not-mask
        nm = [[None] * KT for _ in range(HL)]

        def nm_fetch(h, kt):
            t = pmask.tile([128, N], BF16, tag="nm", name=f"nm{h}_{kt}")
            nc.sync.dma_start(out=t, in_=nmaskT[h, kt * 128:(kt + 1) * 128, :])
            nm[h][kt] = t

        # persistent projection outputs
        qTb = [persist.tile([128, N], BF16, tag=f"qT{p}", name=f"qT{p}")
               for p in range(HP)]
        kTb = [persist.tile([128, N], BF16, tag=f"kT{p}", name=f"kT{p}")
               for p in range(HP)]
        vplus = [persist.tile([128, HL, DH + 1], BF16, tag=f"vp{t}", name=f"vp{t}")
                 for t in range(NT)]
        attnT = persist.tile([128, HP, N], BF16, tag="attnT")

        with tc.tile_pool(name="psS", bufs=2, space="PSUM") as psS, \
             tc.tile_pool(name="psPV", bufs=1, space="PSUM") as psPV:

            # ---- projection filler units (PE work + one DVE copy each) ----
            def qk_unit(which, hp, half, s0):
                def emit(pool, tag="qk"):
                    off = which * HL * DH
                    col = half * HALF + s0
                    ps = pool.tile([128, 1024], F32, tag=tag, name="psqk")
                    for dc in range(DC):
                        nc.tensor.matmul(
                            ps[:, 0:512],
                            lhsT=wb[dc][:, off + hp * 128: off + (hp + 1) * 128],
                            rhs=xTb[dc][:, col:col + 512],
                            start=(dc == 0), stop=(dc == DC - 1))
                    dst = qTb if which == 0 else kTb
                    nc.vector.tensor_copy(dst[hp][:, col:col + 512], ps[:, 0:512])
                return emit

            def v_unit(nt):
                def emit(pool, tag="qk"):
                    voff = 2 * HL * DH
                    ps = pool.tile([128, 1024], F32, tag=tag, name="psv")
                    for dc in range(DC):
                        nc.tensor.matmul(
                            ps[:, 0:HL * DH],
                            lhsT=xTb[dc][:, nt * 128:(nt + 1) * 128],
                            rhs=wb[dc][:, voff: voff + HL * DH],
                            start=(dc == 0), stop=(dc == DC - 1))
                    nc.gpsimd.memset(vplus[nt], 1.0)
                    nc.vector.tensor_copy(
                        vplus[nt][:, :, 0:DH],
                        ps[:, 0:HL * DH].rearrange("p (h d) -> p h d", h=HL))
                return emit

            # filler schedule: {(global half 0..7): {kt: [unit]}}
            FILL = {g: {} for g in range(8)}

            def put(g, kt, unit):
                FILL[g].setdefault(kt, []).append(unit)

            put(0, 0, qk_unit(1, 0, 0, 512))        # k0A cols 512:1024 (kt4-7)
            for i in range(2, 10):                  # vp2..vp9 at kt i-2
                put(0, i - 2, v_unit(i))
            put(0, 4, qk_unit(1, 0, 1, 0))          # k0B (kt8-11)
            put(0, 5, qk_unit(1, 0, 1, 512))        # k0B (kt12-15)
            put(0, 13, qk_unit(0, 0, 1, 0))         # q0B
            put(0, 14, qk_unit(0, 0, 1, 512))
            for i in range(10, NT):                 # vp10..vp15 ride g1 (PV
                put(1, i - 10, v_unit(i))           # lag-12 gives the slack)
            put(1, 10, qk_unit(0, 1, 0, 0))         # q1A
            put(1, 13, qk_unit(0, 1, 0, 512))
            put(2, 0, qk_unit(1, 1, 0, 0))          # k1A
            put(2, 3, qk_unit(1, 1, 0, 512))
            put(2, 6, qk_unit(1, 1, 1, 0))          # k1B
            put(2, 9, qk_unit(1, 1, 1, 512))
            put(2, 12, qk_unit(0, 1, 1, 0))         # q1B
            put(2, 15, qk_unit(0, 1, 1, 512))

            mul_idx = [0]

            def normalize(pvs, h, q0, qw, rel, mul_eng=None):
                hp, ho = h // 2, (h % 2) * 64
                rrow = psmall.tile([1, HALF], F32, tag="rr", name="rrow")
                nc.vector.reciprocal(rrow[:, 0:qw], pvs[DH:DH + 1, rel:rel + qw])
                rb = psmall.tile([64, HALF], F32, tag="rb", name="rb")
                nc.gpsimd.partition_broadcast(rb[:, 0:qw], rrow[:, 0:qw])
                (mul_eng or nc.vector).tensor_mul(
                    attnT[ho:ho + 64, hp, q0:q0 + qw],
                    pvs[0:DH, rel:rel + qw], rb[:, 0:qw])

            def out_proj(nt, pool, copy_eng=None):
                po = pool.tile([128, D], F32, tag="po", name="po")
                for c in range(HP):
                    nc.tensor.matmul(
                        po, lhsT=attnT[:, c, nt * 128:(nt + 1) * 128],
                        rhs=woutb[c], start=(c == 0), stop=(c == HP - 1))
                ob = pout.tile([128, D], BF16, tag="ob", name="ob")
                if copy_eng is None:
                    nc.vector.tensor_copy(ob, po)
                else:
                    copy_eng.copy(ob, po)
                eng = nc.sync if nt % 2 == 1 or nt == 15 else nc.gpsimd
                eng.dma_start(out=out[nt * 128:(nt + 1) * 128, :], in_=ob)

            # ---- flat 128-slot pipeline ----
            pv_of = {}      # global half g -> pv psum tile
            pipe = []       # [(g, kt, pt), ...] PV lag-2 queue

            def emit_pv(g, kt, pt):
                h = g // 2
                if kt == 0:
                    pv_of[g] = psPV.tile([DH + 1, HALF], F32, tag="pv",
                                         name=f"pv{g}")
                pv = pv_of[g]
                for s0 in (0, 512):
                    nc.tensor.matmul(
                        pv[:, s0:s0 + 512],
                        lhsT=vplus[kt][:, h, :],
                        rhs=pt[:, s0:s0 + 512],
                        start=(kt == 0), stop=(kt == KT - 1))

            def finisher(g, pool):
                """pv -> pvs -> normalize for global half g (h<3 or h3-A)."""
                h, half = g // 2, g % 2
                q0 = half * HALF
                pv = pv_of[g]
                pvs = psmall.tile([DH + 1, HALF], F32, tag="pvs", name="pvs")
                nc.vector.tensor_copy(pvs, pv)
                if h == HL - 1:
                    normalize(pvs, h, q0, 512, 0)
                    normalize(pvs, h, q0 + 512, 512, 512)
                else:
                    normalize(pvs, h, q0, HALF, 0, mul_eng=nc.gpsimd)

            SWAPS = {(g_, kt_) for g_ in (2, 3, 4, 5) for kt_ in (7, 15)}

            def emit_exp(g, kt, s, pt):
                if (g, kt) in SWAPS:
                    with tc.high_priority(offset=100):
                        nc.vector.tensor_scalar(
                            pt.bitcast(I16), s, A16 * SCALE, B16,
                            mybir.AluOpType.mult, mybir.AluOpType.add)
                else:
                    nc.scalar.activation(pt, s, Exp, scale=SCALE)

            fin_done = set()

            def flush_one(pool):
                if len(pipe) >= 13:
                    g0, kt0, pt0 = pipe.pop(0)
                    emit_pv(g0, kt0, pt0)
                    if kt0 == KT - 1 and g0 < 7:
                        finisher(g0, pool)
                        fin_done.add(g0)

            with tc.tile_pool(name="psB", bufs=1, space="PSUM") as psB:
                # PE warm-up: ramp the p-state while the first DMAs land
                for _ in range(5):
                    wps = psS.tile([128, 1024], F32, tag="s", name="warm")
                    nc.tensor.matmul(wps[:, 0:512], lhsT=wdum[:, 0:128],
                                     rhs=wdum, start=True, stop=True)
                for kt in range(KT):
                    nm_fetch(0, kt)
                qk_unit(0, 0, 0, 0)(psS, "s")    # q0A
                qk_unit(0, 0, 0, 512)(psB)
                qk_unit(1, 0, 0, 0)(psS, "s")    # k0A cols 0:512 (kt0-3)
                v_unit(0)(psB)
                v_unit(1)(psS, "s")
                for kt in range(KT):
                    nm_fetch(1, kt)

                for g in range(6):
                    h, half = g // 2, g % 2
                    hp, ho = h // 2, (h % 2) * 64
                    q0 = half * HALF
                    if half == 1 and h >= 1:
                        for kt in range(KT):
                            nm_fetch(h + 1, kt)
                    for kt in range(KT):
                        if g >= 3 and kt % 3 == 2:
                            s = psB.tile([128, 1024], F32, tag="qk", name="s3")
                        else:
                            s = psS.tile([128, 1024], F32, tag="s", name="s")
                        for s0 in (0, 512):
                            nc.tensor.matmul(
                                s[:, s0:s0 + 512],
                                lhsT=kTb[hp][ho:ho + 64, kt * 128:(kt + 1) * 128],
                                rhs=qTb[hp][ho:ho + 64, q0 + s0:q0 + s0 + 512],
                                start=True, stop=True)
                        for unit in FILL[g].get(kt, ()):
                            unit(psB)
                        pt = pprob.tile([128, 1024], BF16, tag="pt", name="pt")
                        emit_exp(g, kt, s, pt)
                        with tc.high_priority(offset=100):
                            nc.vector.tensor_mul(
                                pt, pt, nm[h][kt][:, q0:q0 + 1024])
                        mul_idx[0] += 1
                        flush_one(psB)
                        pipe.append((g, kt, pt))

            with tc.tile_pool(name="psO", bufs=2, space="PSUM") as psO:
                po_ride = []   # h3-A out-proj blocks, ridden during h3-B
                for g in (6, 7):
                    h, half = g // 2, g % 2
                    hp, ho = h // 2, (h % 2) * 64
                    q0 = half * HALF
                    for kt in range(KT):
                        s = psS.tile([128, 1024], F32, tag="s", name="s")
                        for s0 in (0, 512):
                            nc.tensor.matmul(
                                s[:, s0:s0 + 512],
                                lhsT=kTb[hp][ho:ho + 64, kt * 128:(kt + 1) * 128],
                                rhs=qTb[hp][ho:ho + 64, q0 + s0:q0 + s0 + 512],
                                start=True, stop=True)
                        pt = pprob.tile([128, 1024], BF16, tag="pt", name="pt")
                        emit_exp(g, kt, s, pt)
                        with tc.high_priority(offset=100):
                            nc.vector.tensor_mul(
                                pt, pt, nm[h][kt][:, q0:q0 + 1024])
                        mul_idx[0] += 1
                        flush_one(psO)
                        if po_ride and 6 in fin_done:
                            out_proj(po_ride.pop(0), psO)
                            if po_ride:
                                out_proj(po_ride.pop(0), psO)
                        pipe.append((g, kt, pt))
                    if g == 6:
                        po_ride = list(range(8))

                # drain the pipeline: pv of h3-B trailing slots
                while pipe:
                    g0, kt0, pt0 = pipe.pop(0)
                    emit_pv(g0, kt0, pt0)
                    if kt0 == KT - 1 and g0 < 7:
                        finisher(g0, psO)
                for nt_ in po_ride:
                    out_proj(nt_, psO)

                # pipelined tail for h3-B: copies fan out to ACT+DVE at
                # once, then 256-column normalize chunks chased by out-proj
                pv = pv_of[7]
                pvs = psmall.tile([DH + 1, HALF], F32, tag="pvs", name="pvs")
                for qq in range(4):
                    cs = slice(qq * 256, (qq + 1) * 256)
                    nc.scalar.copy(pvs[:, cs], pv[:, cs])
                    normalize(pvs, HL - 1, HALF + qq * 256, 256, qq * 256)
                    for j in range(2):
                        nt_ = 8 + 2 * qq + j
                        out_proj(nt_, psO,
                                 copy_eng=nc.scalar if j % 2 == 0 else None)


def build_bass():
    nc = bacc.Bacc("TRN2", target_bir_lowering=False, debug=False, num_devices=8)
    with tile.TileContext(nc) as tc:
        nc.tc = tc
        build_graph(nc)
    nc.compile()
    return nc


def shard_inputs(x, drop_mask, w_qkv, w_out):
    """Host-side sharding: returns in_maps for the 8 cores."""
    x = np.asarray(x, dtype=np.float32)
    drop_mask = np.asarray(drop_mask)
    w_qkv = np.asarray(w_qkv, dtype=np.float32)
    w_out = np.asarray(w_out, dtype=np.float32)
    inner = HEADS * DH
    in_maps = []
    for c in range(8):
        b, g = c // GROUPS, c % GROUPS
        cols = slice(g * HL * DH, (g + 1) * HL * DH)
        wq = w_qkv[:, cols]
        wk = w_qkv[:, inner:][:, cols]
        wv = w_qkv[:, 2 * inner:][:, cols]
        nmT = np.empty((HL, N, N), dtype=NP_BF16)
        for hh in range(HL):
            nmT[hh] = (~drop_mask[b, g * HL + hh]).T.astype(NP_BF16)
        in_maps.append({
            "xT": np.ascontiguousarray(x[b].T).astype(NP_BF16),
            "wqkv": np.ascontiguousarray(
                np.concatenate([wq, wk, wv], axis=1)).astype(NP_BF16),
            "wout": np.ascontiguousarray(
                w_out[g * HL * DH:(g + 1) * HL * DH, :]).astype(NP_BF16),
            "nmaskT": nmT,
        })
    return in_maps


_CACHED_NC = None


def _get_nc():
    global _CACHED_NC
    if _CACHED_NC is None:
        _CACHED_NC = build_bass()
    return _CACHED_NC


def kernel(x, drop_mask, w_qkv, w_out, _trace=False):
    nc = _get_nc()
    in_maps = shard_inputs(x, drop_mask, w_qkv, w_out)
    res = run_bass_kernel_spmd(nc, in_maps, core_ids=list(range(8)), trace=_trace)
    outs = [np.asarray(r["out"], dtype=np.float32) for r in res.results]
    full = np.empty((B, N, D), dtype=np.float32)
    for b in range(B):
        full[b] = outs[b * GROUPS]
        for g in range(1, GROUPS):
            full[b] += outs[b * GROUPS + g]
    kernel.last_results = res
    return full
